# revision 21
# baseline (speedup 1.0000x reference)
"""AttMemoryLayer Trainium2 kernel (8 NeuronCores, batch-parallel).

Math (per batch b):
    scores[s] = sum_d memory[b,s,d] * W[:D]  (+ c_b, c_b = aspect[b]@W[D:] + b)
    p = exp(tanh(scores))          # tanh in [-1,1] => no max-subtraction needed
    out[b] = (sum_s p[s] * memory[b,s,:]) / sum_s p[s]

Distribution: data-parallel over B=64 across 8 cores (8 batches/core),
W/b replicated.  No collectives.

Per-core dataflow (per batch, 4 chunks of 8 s-slices):
  - SWDGE cast-DMA streams each 1MB chunk f32->bf16 as [128, 8, 256]
    (partition = s_outer, free = (s_inner, d)).
  - scores: split across two engines to balance load --
      * N_FUSED slices/chunk: VectorE scalar_tensor_tensor (multiply+reduce
        fused, 1x mode)
      * the rest: VectorE bf16 tensor_tensor multiply (2x mode) then ScalarE
        activation(Copy, accum_out) free-axis reduce
  - ScalarE: tanh (bias = per-batch c, broadcast via a K=1 ones matmul),
    exp with accum_out -> per-(batch,chunk) normalizer partials.
  - PE: 8 accumulating bf16 matmuls per chunk, lhsT = p[:,j] (stationary,
    1 column), rhs = memory slice streams at 1 col/cycle.
  - Normalizer: ones-matmul partition reduce -> X-axis chunk reduce ->
    reciprocal -> one broadcast multiply over the staged [1, 8*256] output.
"""

import sys

for _p in ("/opt/trn_rl_repo",):
    if _p not in sys.path:
        sys.path.append(_p)

import numpy as np

import concourse.bass as bass
import concourse.mybir as mybir
from concourse.tile import TileContext
from concourse.bass_utils import run_bass_kernel_spmd

F32 = mybir.dt.float32
BF16 = mybir.dt.bfloat16

B, S, D = 64, 4096, 256
NCORES = 8
BPC = B // NCORES          # batches per core
SO = 128                   # s_outer (partitions)
SI = S // SO               # s_inner per batch (32)
NCH = 2                    # compute chunks per batch (halves of one DMA)
CJ = SI // NCH             # s-slices per compute chunk (16)


def _split_multi_waits(nc, max_waits=1):
    """This container's walrus build rejects instructions carrying more than
    one sync-wait ("Too many sync wait commands").  Move extra waits onto
    single-wait NoOps inserted immediately before the instruction on the same
    engine; per-engine program order makes this semantics-preserving."""
    cnt = 0
    for bb in nc.main_func.blocks:
        newlist = []
        dirty = False
        for ins in bb.instructions:
            si = ins.sync_info
            if si is not None and si.on_wait and len(si.on_wait) > max_waits:
                waits = list(si.on_wait)
                head, tail = waits[:-max_waits], waits[-max_waits:]
                for w in head:
                    cnt += 1
                    newlist.append(
                        mybir.InstNoOp(
                            name=f"WSPLIT-{cnt}",
                            engine=ins.engine,
                            bass_nofuse=True,
                            sync_info=mybir.SyncInfo(on_wait=[w], on_update=[]),
                        )
                    )
                ins.sync_info = mybir.SyncInfo(
                    on_wait=tail, on_update=list(si.on_update or [])
                )
                dirty = True
            newlist.append(ins)
        if dirty:
            bb.instructions = newlist
    return cnt


def build_nc():
    nc = bass.Bass(trn_type="TRN2")

    MEM = nc.dram_tensor("mem", [BPC, S, D], F32, kind="ExternalInput")
    WMB = nc.dram_tensor("wmb", [128, D], F32, kind="ExternalInput")
    ASPT = nc.dram_tensor("aspt", [128, 2, BPC], F32, kind="ExternalInput")
    WAC = nc.dram_tensor("wac", [128, 2], F32, kind="ExternalInput")
    BSC = nc.dram_tensor("bsc", [1, 1], F32, kind="ExternalInput")
    ONEC = nc.dram_tensor("onec", [128, 1], F32, kind="ExternalInput")
    ONER = nc.dram_tensor("oner", [1, 128], F32, kind="ExternalInput")
    OUT = nc.dram_tensor("out", [1, BPC * D], F32, kind="ExternalOutput")

    mult = mybir.AluOpType.mult
    Act = mybir.ActivationFunctionType

    with TileContext(nc) as tc:
        with (
            tc.tile_pool(name="const", bufs=1) as cpool,
            tc.tile_pool(name="mem", bufs=6) as mpool,
            tc.tile_pool(name="small", bufs=4) as spool,
            tc.tile_pool(name="prods", bufs=3) as prpool,
            tc.tile_pool(name="psums", bufs=1, space="PSUM") as pps,
            tc.tile_pool(name="psumm", bufs=2, space="PSUM") as ppm,
        ):
            # ---- constants / setup -------------------------------------
            wmb16 = cpool.tile([128, D], BF16)
            nc.gpsimd.dma_start(wmb16[:], WMB[:])  # f32 -> bf16 cast DMA
            aspt = cpool.tile([128, 2, BPC], F32)
            nc.sync.dma_start(aspt[:], ASPT[:])
            wac = cpool.tile([128, 2], F32)
            nc.sync.dma_start(wac[:], WAC[:])
            bsc = cpool.tile([1, 1], F32)
            nc.sync.dma_start(bsc[:], BSC[:])
            onec = cpool.tile([128, 1], F32)
            nc.sync.dma_start(onec[:], ONEC[:])
            oner = cpool.tile([1, 128], F32)
            nc.sync.dma_start(oner[:], ONER[:])

            # c_row[1, BPC] = aspect @ Wa + b   (contract d over partitions)
            ps_c = pps.tile([1, BPC], F32)
            nc.tensor.matmul(ps_c[:], lhsT=wac[:, 0:1], rhs=aspt[:, 0, :],
                             start=True, stop=False)
            nc.tensor.matmul(ps_c[:], lhsT=wac[:, 1:2], rhs=aspt[:, 1, :],
                             start=False, stop=True)
            crow = cpool.tile([1, BPC], F32)
            nc.vector.tensor_scalar_add(crow[:], ps_c[:], bsc[0:1, 0:1])

            # c broadcast to all 128 partitions: ones_row.T @ c_row (K=1)
            ps_cb = pps.tile([128, BPC], F32)
            nc.tensor.matmul(ps_cb[:], lhsT=oner[:], rhs=crow[:],
                             start=True, stop=True)
            cb = cpool.tile([128, BPC], F32)
            nc.scalar.copy(cb[:], ps_cb[:])

            lpart = cpool.tile([128, BPC * NCH], F32)    # per-(batch,chunk) l
            stage = cpool.tile([1, BPC, D], F32)         # un-normalized outputs

            # ---- main loop: batches x chunk halves ---------------------
            # The activation+pooling stage for chunk k is emitted while
            # chunk k+1 is being scored, so ScalarE's FIFO never blocks a
            # later chunk's reduces behind a tanh that waits on VectorE.
            out_ps_by_batch = {}
            pend = None

            def flush(pend):
                i, c, scores, bts = pend
                th = spool.tile([128, CJ], F32, tag="th")
                nc.scalar.activation(th[:], scores[:], Act.Tanh,
                                     bias=cb[:, i : i + 1])
                p16 = spool.tile([128, CJ], BF16, tag="p16")
                nc.scalar.activation(p16[:], th[:], Act.Exp)
                nc.vector.reduce_sum(
                    lpart[:, i * NCH + c : i * NCH + c + 1], p16[:],
                    axis=mybir.AxisListType.X,
                )
                out_ps = out_ps_by_batch[i]
                for j in range(CJ):
                    nc.tensor.matmul(
                        out_ps[:], lhsT=p16[:, j : j + 1], rhs=bts[:, j, :],
                        start=(c == 0 and j == 0),
                        stop=(c == NCH - 1 and j == CJ - 1),
                    )
                if c == NCH - 1:
                    # per-batch normalizer: partition-reduce l, reciprocal,
                    # scale the PSUM row on its way into the staging tile
                    ps_l = ppm.tile([1, NCH], F32, tag="ps_l")
                    nc.tensor.matmul(ps_l[:], lhsT=onec[:],
                                     rhs=lpart[:, i * NCH : (i + 1) * NCH],
                                     start=True, stop=True)
                    lsum = spool.tile([1, 1], F32, tag="lsum")
                    nc.vector.reduce_sum(lsum[:], ps_l[:],
                                         axis=mybir.AxisListType.X)
                    lrec = spool.tile([1, 1], F32, tag="lrec")
                    nc.vector.reciprocal(lrec[:], lsum[:])
                    nc.scalar.activation(stage[0:1, i, :], out_ps[:],
                                         Act.Copy, bias=0.0,
                                         scale=lrec[0:1, 0:1])

            for i in range(BPC):
                out_ps_by_batch[i] = ppm.tile([1, D], F32, tag="out_ps", name=f"out_ps_{i}")
                for c in range(NCH):
                    bts = mpool.tile([128, CJ, D], BF16, tag="bt")
                    nc.gpsimd.dma_start(
                        bts[:],
                        MEM[i].rearrange("(so si) d -> so si d", so=SO)[
                            :, c * CJ : (c + 1) * CJ, :
                        ],
                    )
                    # scores[s,j] = sum_d bts[s,j,d] * Wm[d], reduction split
                    # across engines: slices [0:NA) multiply into prodA and
                    # tree-reduce on VectorE; slices [NA:CJ) multiply into
                    # prodB and accum-reduce on ScalarE.
                    NA = CJ - 5
                    prodA = prpool.tile([128, NA, D], BF16, tag="prodA")
                    prodB = prpool.tile([128, CJ - NA, D], BF16, tag="prodB")
                    nc.vector.tensor_tensor(
                        prodB[:], bts[:, NA:CJ, :],
                        wmb16[:, None, :].to_broadcast((128, CJ - NA, D)),
                        mult,
                    )
                    scores = spool.tile([128, CJ], F32, tag="scores")
                    dump = ppm.tile([128, D], F32, tag="dump")
                    for j in range(NA, CJ):
                        nc.scalar.activation(
                            dump[:], prodB[:, j - NA, :], Act.Copy,
                            accum_out=scores[:, j : j + 1],
                        )
                    nc.vector.tensor_tensor(
                        prodA[:], bts[:, 0:NA, :],
                        wmb16[:, None, :].to_broadcast((128, NA, D)), mult,
                    )
                    nc.vector.tensor_add(prodA[:, :, 0:128],
                                         prodA[:, :, 0:128],
                                         prodA[:, :, 128:256])
                    nc.vector.tensor_add(prodA[:, :, 0:64], prodA[:, :, 0:64],
                                         prodA[:, :, 64:128])
                    nc.vector.tensor_add(prodA[:, :, 0:32], prodA[:, :, 0:32],
                                         prodA[:, :, 32:64])
                    nc.vector.reduce_sum(scores[:, 0:NA], prodA[:, :, 0:32],
                                         axis=mybir.AxisListType.X)

                    if pend is not None:
                        flush(pend)
                    pend = (i, c, scores, bts)
            flush(pend)

            nc.sync.dma_start(OUT[:], stage[:].rearrange("p i j -> p (i j)"))

    _split_multi_waits(nc)
    return nc


_NC_CACHE = None


def _get_nc():
    global _NC_CACHE
    if _NC_CACHE is None:
        _NC_CACHE = build_nc()
    return _NC_CACHE


def make_in_maps(aspect, memory, W, b):
    aspect = np.asarray(aspect, dtype=np.float32).reshape(B, D)
    memory = np.ascontiguousarray(np.asarray(memory, dtype=np.float32))
    W = np.asarray(W, dtype=np.float32).reshape(2 * D)
    b = np.asarray(b, dtype=np.float32).reshape(1)

    wmb = np.ascontiguousarray(np.tile(W[:D][None, :], (128, 1)))
    wac = np.ascontiguousarray(W[D:].reshape(2, 128).T)
    bsc = b.reshape(1, 1)
    onec = np.ones((128, 1), dtype=np.float32)
    oner = np.ones((1, 128), dtype=np.float32)

    in_maps = []
    for c in range(NCORES):
        asp = aspect[c * BPC : (c + 1) * BPC]          # [BPC, D]
        aspt = np.ascontiguousarray(
            asp.T.reshape(2, 128, BPC).transpose(1, 0, 2)
        )                                               # [128, 2, BPC]
        in_maps.append(
            {
                "mem": memory[c * BPC : (c + 1) * BPC],
                "wmb": wmb,
                "aspt": aspt,
                "wac": wac,
                "bsc": bsc,
                "onec": onec,
                "oner": oner,
            }
        )
    return in_maps


def run(inputs, trace=False):
    """Returns (out [B, D] float32, exec_time_ns or None)."""
    nc = _get_nc()
    in_maps = make_in_maps(**inputs)
    res = run_bass_kernel_spmd(
        nc, in_maps, core_ids=list(range(NCORES)), trace=trace
    )
    out = np.concatenate(
        [res.results[c]["out"].reshape(BPC, D) for c in range(NCORES)], axis=0
    )
    return out, res.exec_time_ns


def kernel(aspect, memory, W, b):
    out, _ = run(dict(aspect=aspect, memory=memory, W=W, b=b))
    return out


# revision 22
# speedup vs baseline: 1.1260x; 1.1260x over previous
"""AttMemoryLayer Trainium2 kernel (8 NeuronCores, batch-parallel).

Math (per batch b):
    scores[s] = sum_d memory[b,s,d] * W[:D]  (+ c_b, c_b = aspect[b]@W[D:] + b)
    p = exp(tanh(scores))          # tanh in [-1,1] => no max-subtraction needed
    out[b] = (sum_s p[s] * memory[b,s,:]) / sum_s p[s]

Distribution: data-parallel over B=64 across 8 cores (8 batches/core),
W/b replicated.  No collectives.

Per-core dataflow (per batch, 4 chunks of 8 s-slices):
  - SWDGE cast-DMA streams each 1MB chunk f32->bf16 as [128, 8, 256]
    (partition = s_outer, free = (s_inner, d)).
  - scores: split across two engines to balance load --
      * N_FUSED slices/chunk: VectorE scalar_tensor_tensor (multiply+reduce
        fused, 1x mode)
      * the rest: VectorE bf16 tensor_tensor multiply (2x mode) then ScalarE
        activation(Copy, accum_out) free-axis reduce
  - ScalarE: tanh (bias = per-batch c, broadcast via a K=1 ones matmul),
    exp with accum_out -> per-(batch,chunk) normalizer partials.
  - PE: 8 accumulating bf16 matmuls per chunk, lhsT = p[:,j] (stationary,
    1 column), rhs = memory slice streams at 1 col/cycle.
  - Normalizer: ones-matmul partition reduce -> X-axis chunk reduce ->
    reciprocal -> one broadcast multiply over the staged [1, 8*256] output.
"""

import sys

for _p in ("/opt/trn_rl_repo",):
    if _p not in sys.path:
        sys.path.append(_p)

import numpy as np

import concourse.bass as bass
import concourse.mybir as mybir
from concourse.tile import TileContext
from concourse.bass_utils import run_bass_kernel_spmd

F32 = mybir.dt.float32
BF16 = mybir.dt.bfloat16

B, S, D = 64, 4096, 256
NCORES = 8
BPC = B // NCORES          # batches per core
SO = 128                   # s_outer (partitions)
SI = S // SO               # s_inner per batch (32)
NCH = 2                    # compute chunks per batch (halves of one DMA)
CJ = SI // NCH             # s-slices per compute chunk (16)


def _split_multi_waits(nc, max_waits=1):
    """This container's walrus build rejects instructions carrying more than
    one sync-wait ("Too many sync wait commands").  Move extra waits onto
    single-wait NoOps inserted immediately before the instruction on the same
    engine; per-engine program order makes this semantics-preserving."""
    cnt = 0
    for bb in nc.main_func.blocks:
        newlist = []
        dirty = False
        for ins in bb.instructions:
            si = ins.sync_info
            if si is not None and si.on_wait and len(si.on_wait) > max_waits:
                waits = list(si.on_wait)
                head, tail = waits[:-max_waits], waits[-max_waits:]
                for w in head:
                    cnt += 1
                    newlist.append(
                        mybir.InstNoOp(
                            name=f"WSPLIT-{cnt}",
                            engine=ins.engine,
                            bass_nofuse=True,
                            sync_info=mybir.SyncInfo(on_wait=[w], on_update=[]),
                        )
                    )
                ins.sync_info = mybir.SyncInfo(
                    on_wait=tail, on_update=list(si.on_update or [])
                )
                dirty = True
            newlist.append(ins)
        if dirty:
            bb.instructions = newlist
    return cnt


def build_nc():
    nc = bass.Bass(trn_type="TRN2")

    MEM = nc.dram_tensor("mem", [BPC, S, D], F32, kind="ExternalInput")
    WMB = nc.dram_tensor("wmb", [128, D], F32, kind="ExternalInput")
    ASPT = nc.dram_tensor("aspt", [128, 2, BPC], F32, kind="ExternalInput")
    WAC = nc.dram_tensor("wac", [128, 2], F32, kind="ExternalInput")
    BSC = nc.dram_tensor("bsc", [1, 1], F32, kind="ExternalInput")
    ONEC = nc.dram_tensor("onec", [128, 1], F32, kind="ExternalInput")
    ONER = nc.dram_tensor("oner", [1, 128], F32, kind="ExternalInput")
    OUT = nc.dram_tensor("out", [1, BPC * D], F32, kind="ExternalOutput")

    mult = mybir.AluOpType.mult
    Act = mybir.ActivationFunctionType

    with TileContext(nc) as tc:
        with (
            tc.tile_pool(name="const", bufs=1) as cpool,
            tc.tile_pool(name="mem", bufs=6) as mpool,
            tc.tile_pool(name="small", bufs=4) as spool,
            tc.tile_pool(name="prods", bufs=3) as prpool,
            tc.tile_pool(name="psums", bufs=1, space="PSUM") as pps,
            tc.tile_pool(name="psumm", bufs=2, space="PSUM") as ppm,
        ):
            # ---- constants / setup -------------------------------------
            wmb16 = cpool.tile([128, D], BF16)
            nc.gpsimd.dma_start(wmb16[:], WMB[:])  # f32 -> bf16 cast DMA
            aspt = cpool.tile([128, 2, BPC], F32)
            nc.sync.dma_start(aspt[:], ASPT[:])
            wac = cpool.tile([128, 2], F32)
            nc.sync.dma_start(wac[:], WAC[:])
            bsc = cpool.tile([1, 1], F32)
            nc.sync.dma_start(bsc[:], BSC[:])
            onec = cpool.tile([128, 1], F32)
            nc.sync.dma_start(onec[:], ONEC[:])
            oner = cpool.tile([1, 128], F32)
            nc.sync.dma_start(oner[:], ONER[:])

            # c_row[1, BPC] = aspect @ Wa + b   (contract d over partitions)
            ps_c = pps.tile([1, BPC], F32)
            nc.tensor.matmul(ps_c[:], lhsT=wac[:, 0:1], rhs=aspt[:, 0, :],
                             start=True, stop=False)
            nc.tensor.matmul(ps_c[:], lhsT=wac[:, 1:2], rhs=aspt[:, 1, :],
                             start=False, stop=True)
            crow = cpool.tile([1, BPC], F32)
            nc.vector.tensor_scalar_add(crow[:], ps_c[:], bsc[0:1, 0:1])

            # c broadcast to all 128 partitions: ones_row.T @ c_row (K=1)
            ps_cb = pps.tile([128, BPC], F32)
            nc.tensor.matmul(ps_cb[:], lhsT=oner[:], rhs=crow[:],
                             start=True, stop=True)
            cb = cpool.tile([128, BPC], F32)
            nc.scalar.copy(cb[:], ps_cb[:])

            lpart = cpool.tile([128, BPC * NCH], F32)    # per-(batch,chunk) l
            stage = cpool.tile([1, BPC, D], F32)         # un-normalized outputs

            # ---- main loop: batches x chunk halves ---------------------
            for i in range(BPC):
                out_ps = ppm.tile([1, D], F32, tag="out_ps")
                for c in range(NCH):
                    bts = mpool.tile([128, CJ, D], BF16, tag="bt")
                    nc.gpsimd.dma_start(
                        bts[:],
                        MEM[i].rearrange("(so si) d -> so si d", so=SO)[
                            :, c * CJ : (c + 1) * CJ, :
                        ],
                    )
                    # scores[s,j] = sum_d bts[s,j,d] * Wm[d], reduction split
                    # across engines: slices [0:NA) multiply into prodA and
                    # tree-reduce on VectorE; slices [NA:CJ) multiply into
                    # prodB and accum-reduce on ScalarE.
                    NA = CJ - 5
                    prodA = prpool.tile([128, NA, D], BF16, tag="prodA")
                    prodB = prpool.tile([128, CJ - NA, D], BF16, tag="prodB")
                    nc.vector.tensor_tensor(
                        prodB[:], bts[:, NA:CJ, :],
                        wmb16[:, None, :].to_broadcast((128, CJ - NA, D)),
                        mult,
                    )
                    scores = spool.tile([128, CJ], F32, tag="scores")
                    dump = ppm.tile([128, D], F32, tag="dump")
                    for j in range(NA, CJ):
                        nc.scalar.activation(
                            dump[:], prodB[:, j - NA, :], Act.Copy,
                            accum_out=scores[:, j : j + 1],
                        )
                    nc.vector.tensor_tensor(
                        prodA[:], bts[:, 0:NA, :],
                        wmb16[:, None, :].to_broadcast((128, NA, D)), mult,
                    )
                    nc.vector.tensor_add(prodA[:, :, 0:128],
                                         prodA[:, :, 0:128],
                                         prodA[:, :, 128:256])
                    nc.vector.tensor_add(prodA[:, :, 0:64], prodA[:, :, 0:64],
                                         prodA[:, :, 64:128])
                    nc.vector.tensor_add(prodA[:, :, 0:32], prodA[:, :, 0:32],
                                         prodA[:, :, 32:64])
                    nc.vector.reduce_sum(scores[:, 0:NA], prodA[:, :, 0:32],
                                         axis=mybir.AxisListType.X)

                    th = spool.tile([128, CJ], F32, tag="th")
                    nc.scalar.activation(th[:], scores[:], Act.Tanh,
                                         bias=cb[:, i : i + 1])
                    p16 = spool.tile([128, CJ], BF16, tag="p16")
                    nc.scalar.activation(p16[:], th[:], Act.Exp)
                    nc.vector.reduce_sum(
                        lpart[:, i * NCH + c : i * NCH + c + 1], p16[:],
                        axis=mybir.AxisListType.X,
                    )

                    for j in range(CJ):
                        nc.tensor.matmul(
                            out_ps[:], lhsT=p16[:, j : j + 1],
                            rhs=bts[:, j, :],
                            start=(c == 0 and j == 0),
                            stop=(c == NCH - 1 and j == CJ - 1),
                        )
                # per-batch normalizer: partition-reduce l, reciprocal,
                # and scale the PSUM row on its way into the staging tile
                ps_l = ppm.tile([1, NCH], F32, tag="ps_l")
                nc.tensor.matmul(ps_l[:], lhsT=onec[:],
                                 rhs=lpart[:, i * NCH : (i + 1) * NCH],
                                 start=True, stop=True)
                lsum = spool.tile([1, 1], F32, tag="lsum")
                nc.vector.reduce_sum(lsum[:], ps_l[:],
                                     axis=mybir.AxisListType.X)
                lrec = spool.tile([1, 1], F32, tag="lrec")
                nc.vector.reciprocal(lrec[:], lsum[:])
                nc.scalar.activation(stage[0:1, i, :], out_ps[:], Act.Copy,
                                     bias=0.0, scale=lrec[0:1, 0:1])

            nc.sync.dma_start(OUT[:], stage[:].rearrange("p i j -> p (i j)"))

    _split_multi_waits(nc)
    return nc


_NC_CACHE = None


def _get_nc():
    global _NC_CACHE
    if _NC_CACHE is None:
        _NC_CACHE = build_nc()
    return _NC_CACHE


def make_in_maps(aspect, memory, W, b):
    aspect = np.asarray(aspect, dtype=np.float32).reshape(B, D)
    memory = np.ascontiguousarray(np.asarray(memory, dtype=np.float32))
    W = np.asarray(W, dtype=np.float32).reshape(2 * D)
    b = np.asarray(b, dtype=np.float32).reshape(1)

    wmb = np.ascontiguousarray(np.tile(W[:D][None, :], (128, 1)))
    wac = np.ascontiguousarray(W[D:].reshape(2, 128).T)
    bsc = b.reshape(1, 1)
    onec = np.ones((128, 1), dtype=np.float32)
    oner = np.ones((1, 128), dtype=np.float32)

    in_maps = []
    for c in range(NCORES):
        asp = aspect[c * BPC : (c + 1) * BPC]          # [BPC, D]
        aspt = np.ascontiguousarray(
            asp.T.reshape(2, 128, BPC).transpose(1, 0, 2)
        )                                               # [128, 2, BPC]
        in_maps.append(
            {
                "mem": memory[c * BPC : (c + 1) * BPC],
                "wmb": wmb,
                "aspt": aspt,
                "wac": wac,
                "bsc": bsc,
                "onec": onec,
                "oner": oner,
            }
        )
    return in_maps


def run(inputs, trace=False):
    """Returns (out [B, D] float32, exec_time_ns or None)."""
    nc = _get_nc()
    in_maps = make_in_maps(**inputs)
    res = run_bass_kernel_spmd(
        nc, in_maps, core_ids=list(range(NCORES)), trace=trace
    )
    out = np.concatenate(
        [res.results[c]["out"].reshape(BPC, D) for c in range(NCORES)], axis=0
    )
    return out, res.exec_time_ns


def kernel(aspect, memory, W, b):
    out, _ = run(dict(aspect=aspect, memory=memory, W=W, b=b))
    return out
